# revision 7
# baseline (speedup 1.0000x reference)
"""DirectionalContrastiveLoss on 8 TRN2 NeuronCores (Bass/Tile), v2.

Data-parallel over the N=16384 anchor rows (2048 rows/core); the 4000-row
memory bank is replicated (padded to 4096 columns with zero features).

Device algorithm:
- PSUM holds SC*(sim - pos + B0) per [128-row tile x 4096 mem cols],
  SC = 184.664 = 2^7/ln2, B0 = 88.03 - schraudolph shift:
  * features as fp8e4m3 DoubleRow matmuls (K=256 in one pass, both sides
    scaled by sqrt(SC/TEMP) = 42.97),
  * label mask as bf16 -SC*1000*onehot(label) x onehot(mem_label)
    matmuls on per-chunk 32-row PE tile positions (4-way concurrent),
  * per-row bias SC*(B0 - pos) via a 22nd one-hot row (x valid-col
    indicator, so pad columns stay exactly 0).
- Split exp-sum: ACT exps cols [0,ACOLS) (scale=1/SC, bias=-B0) with
  accum_out; DVE turns cols [ACOLS,4096) into int16 = clamp(psum, 0,
  32512) whose bf16 bitcast IS Schraudolph exp (no scale needed - it's
  in the matmul); DVE+GpSimd reduce-sum the bitcast tile.
- Rows with sim-pos > ~89.5 saturate to huge-finite/inf -> the host's
  -log(1/(S+1+eps)+eps) clamps them to the reference's 18.42 value
  (dead rows), exactly matching the fp32 reference semantics.
Device exports per-row partial sums [128, 6*NT]; the host computes the
-log epilogue, masks, and the final scalar (exact fp64).
"""
from contextlib import ExitStack

import numpy as np
import ml_dtypes

TEMP = 0.1
POS_THRESH = 0.7
EPS = 1e-8
N, C, M, NLAB = 16384, 256, 4000, 21
MP = 4096                  # memory columns padded
NCORES = 8
RPC = N // NCORES          # 2048 rows per core
NT = RPC // 128            # 16 n-tiles per core
CHUNK = 256                # psum region granularity (DR matmul out cols)
NCH = MP // CHUNK          # 16 chunks per n-tile

SC = 128.0 / np.log(2.0)            # 184.6635
SHIFT = 0.0579                      # schraudolph centering (in ln2 units)
B0 = (127.0 - SHIFT) * np.log(2.0)  # 87.9896  (psum bias; ACT cancels it)
SQ = float(np.sqrt(SC / TEMP))      # 42.9725  (fp8 per-side scale)
CAPV = 32512.0                      # 0x7F00 -> bf16 1.66e38 (huge finite)

ACOLS = 3072               # psum cols exp'd by ACT (bank-aligned w/ B)
BCOLS = MP - ACOLS         # cols converted by DVE (schraudolph)

_cache = {}


def _build():
    import concourse.bacc as bacc
    import concourse.tile as tile
    from concourse import mybir

    f32 = mybir.dt.float32
    bf16 = mybir.dt.bfloat16
    f8 = mybir.dt.float8e4
    i16 = mybir.dt.int16
    Alu = mybir.AluOpType
    Act = mybir.ActivationFunctionType
    X = mybir.AxisListType.X
    DR = mybir.MatmulPerfMode.DoubleRow

    nc = bacc.Bacc(None)

    # DRAM params. ext: [128, NT, 2, 128] fp8 DR layout (k = i*128 + p).
    ext1_d = nc.declare_dram_parameter("ext1", [128, NT * 2 * 128], f8, isOutput=False)
    ext2_d = nc.declare_dram_parameter("ext2", [128, NT * 2 * 128], f8, isOutput=False)
    mem_d = nc.declare_dram_parameter("extmem", [128, 2 * MP], f8, isOutput=False)
    eqa1_d = nc.declare_dram_parameter("eqanc1", [128, RPC], bf16, isOutput=False)
    eqa2_d = nc.declare_dram_parameter("eqanc2", [128, RPC], bf16, isOutput=False)
    eqm_d = nc.declare_dram_parameter("eqmem", [128, MP], bf16, isOutput=False)
    out_d = nc.declare_dram_parameter("out", [128, 6 * NT], f32, isOutput=True)

    T0C = 4  # tiles in the startup DMA chunk

    with tile.TileContext(nc) as tc, ExitStack() as ctx:
        consts = ctx.enter_context(tc.tile_pool(name="consts", bufs=1))
        psum = ctx.enter_context(tc.tile_pool(name="psum", bufs=1, space="PSUM"))
        sb = ctx.enter_context(tc.tile_pool(name="sb", bufs=2))

        # ---- resident inputs, ordered by first use ----
        # (branch 0, tile 0 needs: ext1 t<4, mem, eqa1 t<4, eqm)
        ext1 = consts.tile([128, NT, 2, 128], f8, tag="ext1", name="ext1")
        ext2 = consts.tile([128, NT, 2, 128], f8, tag="ext2", name="ext2")
        mem8 = consts.tile([128, 2, MP], f8, tag="mem8", name="mem8")
        eqa1 = consts.tile([128, RPC], bf16, tag="eqa1", name="eqa1")
        eqa2 = consts.tile([128, RPC], bf16, tag="eqa2", name="eqa2")
        eqm = consts.tile([128, MP], bf16, tag="eqm", name="eqm")

        e1_r = ext1_d[:].rearrange("p (t i m) -> p t i m", i=2, m=128)
        e2_r = ext2_d[:].rearrange("p (t i m) -> p t i m", i=2, m=128)
        m_r = mem_d[:].rearrange("p (i j) -> p i j", i=2)

        nc.sync.dma_start(out=ext1[:, 0:T0C], in_=e1_r[:, 0:T0C])
        nc.sync.dma_start(out=mem8[:], in_=m_r[:])
        nc.sync.dma_start(out=eqa1[:, 0 : T0C * 128], in_=eqa1_d[:, 0 : T0C * 128])
        nc.sync.dma_start(out=eqm[:], in_=eqm_d[:])
        nc.sync.dma_start(out=ext1[:, T0C:NT], in_=e1_r[:, T0C:NT])
        nc.sync.dma_start(out=eqa1[:, T0C * 128 :], in_=eqa1_d[:, T0C * 128 :])
        nc.sync.dma_start(out=ext2[:], in_=e2_r[:])
        nc.sync.dma_start(out=eqa2[:], in_=eqa2_d[:])

        biasA = consts.tile([128, 1], f32, tag="biasA", name="biasA")
        nc.vector.memset(biasA[:], -B0)

        SSA = consts.tile([128, 2, NT], f32, tag="SSA", name="SSA")
        SSD = consts.tile([128, 2, NT], f32, tag="SSD", name="SSD")
        SSP = consts.tile([128, 2, NT], f32, tag="SSP", name="SSP")
        nc.gpsimd.memset(SSP[:], 0.0)

        for b, (ext, eqa) in enumerate([(ext1, eqa1), (ext2, eqa2)]):
            for t in range(NT):
                tc0 = t * 128
                lhsT = ext[:, t]                    # [128, 2, 128] fp8
                pB = psum.tile([128, BCOLS], f32, tag="pB", name=f"pB{b}_{t}")
                pA = psum.tile([128, ACOLS], f32, tag="pA", name=f"pA{b}_{t}")

                # B region first (cols ACOLS..4096), then A: DVE starts
                # early and B banks recycle while ACT drains A.
                for reg, (p0, w) in enumerate([(ACOLS, BCOLS), (0, ACOLS)]):
                    pt = pB if reg == 0 else pA
                    for ci in range(w // CHUNK):
                        c0 = p0 + ci * CHUNK
                        o0 = ci * CHUNK
                        nc.tensor.matmul(
                            pt[:, o0 : o0 + CHUNK],
                            lhsT,
                            mem8[:, :, c0 : c0 + CHUNK],
                            start=True,
                            stop=False,
                            perf_mode=DR,
                        )
                        u = ci % 4
                        nc.tensor.matmul(
                            pt[:, o0 : o0 + CHUNK],
                            eqa[32 * u : 32 * u + NLAB + 1, tc0 : tc0 + 128],
                            eqm[32 * u : 32 * u + NLAB + 1, c0 : c0 + CHUNK],
                            start=False,
                            stop=True,
                            tile_position=(32 * u, 0),
                        )

                # DVE: int16 = clamp(psum, 0, 32512); bitcast bf16 = exp
                ex = sb.tile([128, BCOLS], i16, tag="ex", name=f"ex{b}_{t}")
                nc.vector.tensor_scalar(
                    out=ex[:],
                    in0=pB[:],
                    scalar1=0.0,
                    scalar2=CAPV,
                    op0=Alu.max,
                    op1=Alu.min,
                )
                exb = ex[:].bitcast(bf16)
                nc.vector.reduce_sum(
                    out=SSD[:, b, t : t + 1], in_=exb[:], axis=X
                )

                # ACT: exp((psum/SC) - B0), accumulate row sum
                nc.scalar.activation(
                    out=pA[:],
                    in_=pA[:],
                    func=Act.Exp,
                    bias=biasA[:],
                    scale=float(1.0 / SC),
                    accum_out=SSA[:, b, t : t + 1],
                )

        nc.sync.dma_start(out=out_d[:, 0 : 2 * NT], in_=SSA[:].rearrange("p b t -> p (b t)"))
        nc.sync.dma_start(out=out_d[:, 2 * NT : 4 * NT], in_=SSD[:].rearrange("p b t -> p (b t)"))
        nc.sync.dma_start(out=out_d[:, 4 * NT : 6 * NT], in_=SSP[:].rearrange("p b t -> p (b t)"))

    nc.finalize()
    return nc


def _host_prep(inputs):
    bf = ml_dtypes.bfloat16
    f8 = ml_dtypes.float8_e4m3
    f1 = np.ascontiguousarray(np.asarray(inputs["output_feat1"], np.float32))
    f2 = np.ascontiguousarray(np.asarray(inputs["output_feat2"], np.float32))
    l1 = np.asarray(inputs["pseudo_label1"], np.int32)
    l2 = np.asarray(inputs["pseudo_label2"], np.int32)
    ul1 = np.asarray(inputs["output_ul1"], np.float32)
    ul2 = np.asarray(inputs["output_ul2"], np.float32)
    i1 = np.asarray(inputs["selected_idx1"], np.int64)
    i2 = np.asarray(inputs["selected_idx2"], np.int64)

    b, c, h, w = ul1.shape
    u1 = ul1.transpose(0, 2, 3, 1).reshape(b * h * w, c)
    u2 = ul2.transpose(0, 2, 3, 1).reshape(b * h * w, c)
    mem = np.concatenate([u1[i1], u2[i2]], axis=0)               # [M, C]
    memlab = np.concatenate([l1[i1], l2[i2]], axis=0)            # [M]

    pos = (f1 * f2).sum(axis=1, dtype=np.float64) / TEMP         # [N] exact

    # memory fp8 DR layout: mem8[p, i, j] = mem[j, 128i+p] * SQ
    extmem = np.zeros((128, 2, MP), np.float32)
    mt = (mem.T * SQ)                                            # [C, M]
    extmem[:, 0, :M] = mt[0:128]
    extmem[:, 1, :M] = mt[128:256]
    extmem = extmem.astype(f8).reshape(128, 2 * MP)

    # mask memory side: rows 32u+i = onehot(memlab==i); row 32u+21 = valid
    lab_eye = np.arange(NLAB, dtype=np.int32)
    eqmem = np.zeros((128, MP), np.float32)
    oh_mem = (memlab[None, :] == lab_eye[:, None]).astype(np.float32)
    for u in range(4):
        eqmem[32 * u : 32 * u + NLAB, :M] = oh_mem
        eqmem[32 * u + NLAB, :M] = 1.0
    eqmem = eqmem.astype(bf)

    def eq_anchor(lab, pos_sl):
        out = np.zeros((128, lab.shape[0]), np.float32)
        oh = (-SC * 1000.0) * (lab[None, :] == lab_eye[:, None])
        brow = SC * (B0 - pos_sl)
        for u in range(4):
            out[32 * u : 32 * u + NLAB] = oh
            out[32 * u + NLAB] = brow
        return out.astype(bf)

    def pack_ext(x):   # [RPC, C] fp32 -> [128, NT*2*128] fp8 DR layout
        # ext[p, t, i, m] = x[t*128 + m, i*128 + p] * SQ
        v = (x * SQ).reshape(NT, 128, 2, 128)        # [t, m, i, p]
        v = v.transpose(3, 0, 2, 1)                  # [p, t, i, m]
        return np.ascontiguousarray(v).astype(f8).reshape(128, NT * 2 * 128)

    in_maps = []
    for cix in range(NCORES):
        sl = slice(cix * RPC, (cix + 1) * RPC)
        in_maps.append({
            "ext1": pack_ext(f1[sl]),
            "ext2": pack_ext(f2[sl]),
            "extmem": extmem,
            "eqanc1": np.ascontiguousarray(eq_anchor(l1[sl], pos[sl])),
            "eqanc2": np.ascontiguousarray(eq_anchor(l2[sl], pos[sl])),
            "eqmem": eqmem,
        })
    return in_maps, pos


def _finalize(results, inputs):
    g1 = np.asarray(inputs["pseudo_logits1"], np.float64)
    g2 = np.asarray(inputs["pseudo_logits2"], np.float64)

    # device partials -> S per row, ordered [core, tile, lane]
    S = np.zeros((2, N), np.float64)
    for cix, r in enumerate(results):
        o = np.asarray(r["out"], np.float64)         # [128, 6*NT]
        for b in range(2):
            ssa = o[:, b * NT : (b + 1) * NT]
            ssd = o[:, 2 * NT + b * NT : 2 * NT + (b + 1) * NT]
            ssp = o[:, 4 * NT + b * NT : 4 * NT + (b + 1) * NT]
            st = ssa + ssd + ssp                     # [128 lanes, NT]
            # row (cix*RPC + t*128 + lane) <- st[lane, t]
            S[b, cix * RPC : (cix + 1) * RPC] = st.T.reshape(RPC)

    S = np.nan_to_num(S, nan=np.inf, posinf=np.inf, neginf=0.0)
    with np.errstate(divide="ignore", over="ignore"):
        sig = 1.0 / (S + 1.0 + EPS)
        lam = -np.log(sig + EPS)                     # per-row loss term

    m1 = ((g2 > POS_THRESH) & (g1 < g2)).astype(np.float64)
    m2 = ((g1 > POS_THRESH) & (g2 < g1)).astype(np.float64)
    loss = (lam[0] * m1).sum() / (m1.sum() + 1e-12) + \
           (lam[1] * m2).sum() / (m2.sum() + 1e-12)
    return np.float32(loss)


def _run(inputs, trace=False):
    from concourse.bass_utils import run_bass_kernel_spmd

    if "nc" not in _cache:
        _cache["nc"] = _build()
    in_maps, _pos = _host_prep(inputs)
    res = run_bass_kernel_spmd(
        _cache["nc"], in_maps, list(range(NCORES)), trace=trace
    )
    return _finalize(res.results, inputs), res


def kernel(**inputs):
    out, _ = _run(inputs)
    return out


def kernel_with_profile(**inputs):
    out, res = _run(inputs, trace=True)
    return out, res


# revision 12
# speedup vs baseline: 1.8003x; 1.8003x over previous
"""DirectionalContrastiveLoss on 8 TRN2 NeuronCores (Bass/Tile), v2.

Data-parallel over the N=16384 anchor rows (2048 rows/core); the 4000-row
memory bank is replicated (padded to 4096 columns with zero features).

Device algorithm:
- PSUM holds SC*(sim - pos + B0) per [128-row tile x 4096 mem cols],
  SC = 184.664 = 2^7/ln2, B0 = 88.03 - schraudolph shift:
  * features as fp8e4m3 DoubleRow matmuls (K=256 in one pass, both sides
    scaled by sqrt(SC/TEMP) = 42.97),
  * label mask as bf16 -SC*1000*onehot(label) x onehot(mem_label)
    matmuls on per-chunk 32-row PE tile positions (4-way concurrent),
  * per-row bias SC*(B0 - pos) via a 22nd one-hot row (x valid-col
    indicator, so pad columns stay exactly 0).
- Split exp-sum: ACT exps cols [0,ACOLS) (scale=1/SC, bias=-B0) with
  accum_out; DVE turns cols [ACOLS,4096) into int16 = clamp(psum, 0,
  32512) whose bf16 bitcast IS Schraudolph exp (no scale needed - it's
  in the matmul); DVE+GpSimd reduce-sum the bitcast tile.
- Rows with sim-pos > ~89.5 saturate to huge-finite/inf -> the host's
  -log(1/(S+1+eps)+eps) clamps them to the reference's 18.42 value
  (dead rows), exactly matching the fp32 reference semantics.
Device exports per-row partial sums [128, 6*NT]; the host computes the
-log epilogue, masks, and the final scalar (exact fp64).
"""
from contextlib import ExitStack

import numpy as np
import ml_dtypes

TEMP = 0.1
POS_THRESH = 0.7
EPS = 1e-8
N, C, M, NLAB = 16384, 256, 4000, 21
MP = 4096                  # memory columns padded
NCORES = 8
RPC = N // NCORES          # 2048 rows per core
NT = RPC // 128            # 16 n-tiles per core
CHUNK = 256                # psum region granularity (DR matmul out cols)
NCH = MP // CHUNK          # 16 chunks per n-tile

SC = 128.0 / np.log(2.0)            # 184.6635
SHIFT = 0.0579                      # schraudolph centering (in ln2 units)
B0 = (127.0 - SHIFT) * np.log(2.0)  # 87.9896  (psum bias; ACT cancels it)
SQ = float(np.sqrt(SC / TEMP))      # 42.9725  (bf16 per-side scale)
CAPV = 32512.0                      # 0x7F00 -> bf16 1.66e38 (huge finite)
J = 512                             # psum chunk width (bank)

ACOLS = 3072               # psum cols exp'd by ACT (bank-aligned w/ B)
BCOLS = MP - ACOLS         # cols converted by DVE (schraudolph)

_cache = {}


def _build():
    import concourse.bacc as bacc
    import concourse.tile as tile
    from concourse import mybir

    f32 = mybir.dt.float32
    bf16 = mybir.dt.bfloat16
    f8 = mybir.dt.float8e4
    i16 = mybir.dt.int16
    Alu = mybir.AluOpType
    Act = mybir.ActivationFunctionType
    X = mybir.AxisListType.X
    DR = mybir.MatmulPerfMode.DoubleRow

    nc = bacc.Bacc(None)

    # DRAM params. ext: [C, RPC] bf16 (2 K-tiles of 128 partitions).
    ext1_d = nc.declare_dram_parameter("ext1", [C, RPC], bf16, isOutput=False)
    ext2_d = nc.declare_dram_parameter("ext2", [C, RPC], bf16, isOutput=False)
    mem_d = nc.declare_dram_parameter("extmem", [C, MP], bf16, isOutput=False)
    eqa1_d = nc.declare_dram_parameter("eqanc1", [128, RPC], bf16, isOutput=False)
    eqa2_d = nc.declare_dram_parameter("eqanc2", [128, RPC], bf16, isOutput=False)
    eqm_d = nc.declare_dram_parameter("eqmem", [128, MP], bf16, isOutput=False)
    out_d = nc.declare_dram_parameter("out", [128, 6 * NT], f32, isOutput=True)

    T0C = 4  # tiles in the startup DMA chunk

    with tile.TileContext(nc) as tc, ExitStack() as ctx:
        consts = ctx.enter_context(tc.tile_pool(name="consts", bufs=1))
        psum = ctx.enter_context(tc.tile_pool(name="psum", bufs=1, space="PSUM"))
        sb = ctx.enter_context(tc.tile_pool(name="sb", bufs=2))

        # ---- resident inputs, ordered by first use ----
        # (branch 0, tile 0 needs: ext1 t<4, mem, eqa1 t<4, eqm)
        e1_k = [
            consts.tile([128, RPC], bf16, tag=f"e1_{i}", name=f"e1_{i}")
            for i in range(2)
        ]
        e2_k = [
            consts.tile([128, RPC], bf16, tag=f"e2_{i}", name=f"e2_{i}")
            for i in range(2)
        ]
        mem_k = [
            consts.tile([128, MP], bf16, tag=f"mem_{i}", name=f"mem_{i}")
            for i in range(2)
        ]
        eqa1 = consts.tile([128, RPC], bf16, tag="eqa1", name="eqa1")
        eqa2 = consts.tile([128, RPC], bf16, tag="eqa2", name="eqa2")
        eqm = consts.tile([128, MP], bf16, tag="eqm", name="eqm")

        t0c = T0C * 128
        for i in range(2):
            nc.sync.dma_start(
                out=e1_k[i][:, 0:t0c], in_=ext1_d[128 * i : 128 * i + 128, 0:t0c]
            )
            nc.sync.dma_start(out=mem_k[i][:], in_=mem_d[128 * i : 128 * i + 128, :])
        nc.sync.dma_start(out=eqa1[:, 0:t0c], in_=eqa1_d[:, 0:t0c])
        nc.sync.dma_start(out=eqm[:], in_=eqm_d[:])
        for i in range(2):
            nc.sync.dma_start(
                out=e1_k[i][:, t0c:], in_=ext1_d[128 * i : 128 * i + 128, t0c:]
            )
        nc.sync.dma_start(out=eqa1[:, t0c:], in_=eqa1_d[:, t0c:])
        for i in range(2):
            nc.sync.dma_start(out=e2_k[i][:], in_=ext2_d[128 * i : 128 * i + 128, :])
        nc.sync.dma_start(out=eqa2[:], in_=eqa2_d[:])

        biasA = consts.tile([128, 1], f32, tag="biasA", name="biasA")
        nc.vector.memset(biasA[:], -B0)

        SSA = consts.tile([128, 2, NT], f32, tag="SSA", name="SSA")
        SSD = consts.tile([128, 2, NT], f32, tag="SSD", name="SSD")
        SSP = consts.tile([128, 2, NT], f32, tag="SSP", name="SSP")
        nc.gpsimd.memset(SSP[:], 0.0)

        for b, (ekt, eqa) in enumerate([(e1_k, eqa1), (e2_k, eqa2)]):
            for t in range(NT):
                tc0 = t * 128
                pB = psum.tile([128, BCOLS], f32, tag="pB", name=f"pB{b}_{t}")
                pA = psum.tile([128, ACOLS], f32, tag="pA", name=f"pA{b}_{t}")

                # B region first (cols ACOLS..4096), then A: DVE starts
                # early and B banks recycle while ACT drains A.
                for reg, (p0, w) in enumerate([(ACOLS, BCOLS), (0, ACOLS)]):
                    pt = pB if reg == 0 else pA
                    for kt in range(2):
                        lhsT = ekt[kt][:, tc0 : tc0 + 128]
                        for ci in range(w // J):
                            o0 = ci * J
                            c0 = p0 + o0
                            nc.tensor.matmul(
                                pt[:, o0 : o0 + J],
                                lhsT,
                                mem_k[kt][:, c0 : c0 + J],
                                start=(kt == 0),
                                stop=False,
                            )
                    for ci in range(w // J):
                        o0 = ci * J
                        c0 = p0 + o0
                        u = ci % 4
                        nc.tensor.matmul(
                            pt[:, o0 : o0 + J],
                            eqa[32 * u : 32 * u + NLAB + 1, tc0 : tc0 + 128],
                            eqm[32 * u : 32 * u + NLAB + 1, c0 : c0 + J],
                            start=False,
                            stop=True,
                            tile_position=(32 * u, 0),
                        )

                # DVE: int16 = clamp(psum, 0, 32512); bitcast bf16 = exp
                ex = sb.tile([128, BCOLS], i16, tag="ex", name=f"ex{b}_{t}")
                nc.vector.tensor_scalar(
                    out=ex[:],
                    in0=pB[:],
                    scalar1=0.0,
                    scalar2=CAPV,
                    op0=Alu.max,
                    op1=Alu.min,
                )
                exb = ex[:].bitcast(bf16)
                nc.vector.reduce_sum(
                    out=SSD[:, b, t : t + 1], in_=exb[:], axis=X
                )

                # ACT: exp((psum/SC) - B0), accumulate row sum
                nc.scalar.activation(
                    out=pA[:],
                    in_=pA[:],
                    func=Act.Exp,
                    bias=biasA[:],
                    scale=float(1.0 / SC),
                    accum_out=SSA[:, b, t : t + 1],
                )

        nc.sync.dma_start(out=out_d[:, 0 : 2 * NT], in_=SSA[:].rearrange("p b t -> p (b t)"))
        nc.sync.dma_start(out=out_d[:, 2 * NT : 4 * NT], in_=SSD[:].rearrange("p b t -> p (b t)"))
        nc.sync.dma_start(out=out_d[:, 4 * NT : 6 * NT], in_=SSP[:].rearrange("p b t -> p (b t)"))

    nc.finalize()
    return nc


def _host_prep(inputs):
    bf = ml_dtypes.bfloat16
    f8 = ml_dtypes.float8_e4m3
    f1 = np.ascontiguousarray(np.asarray(inputs["output_feat1"], np.float32))
    f2 = np.ascontiguousarray(np.asarray(inputs["output_feat2"], np.float32))
    l1 = np.asarray(inputs["pseudo_label1"], np.int32)
    l2 = np.asarray(inputs["pseudo_label2"], np.int32)
    ul1 = np.asarray(inputs["output_ul1"], np.float32)
    ul2 = np.asarray(inputs["output_ul2"], np.float32)
    i1 = np.asarray(inputs["selected_idx1"], np.int64)
    i2 = np.asarray(inputs["selected_idx2"], np.int64)

    b, c, h, w = ul1.shape
    u1 = ul1.transpose(0, 2, 3, 1).reshape(b * h * w, c)
    u2 = ul2.transpose(0, 2, 3, 1).reshape(b * h * w, c)
    mem = np.concatenate([u1[i1], u2[i2]], axis=0)               # [M, C]
    memlab = np.concatenate([l1[i1], l2[i2]], axis=0)            # [M]

    pos = (f1 * f2).sum(axis=1, dtype=np.float64) / TEMP         # [N] exact

    extmem = np.zeros((C, MP), np.float32)
    extmem[:, :M] = mem.T * SQ
    extmem = extmem.astype(bf)                                   # [C, MP]

    # mask memory side: rows 32u+i = onehot(memlab==i); row 32u+21 = valid
    lab_eye = np.arange(NLAB, dtype=np.int32)
    eqmem = np.zeros((128, MP), np.float32)
    oh_mem = (memlab[None, :] == lab_eye[:, None]).astype(np.float32)
    for u in range(4):
        eqmem[32 * u : 32 * u + NLAB, :M] = oh_mem
        eqmem[32 * u + NLAB, :M] = 1.0
    eqmem = eqmem.astype(bf)

    def eq_anchor(lab, pos_sl):
        out = np.zeros((128, lab.shape[0]), np.float32)
        oh = (-SC * 1000.0) * (lab[None, :] == lab_eye[:, None])
        brow = SC * (B0 - pos_sl)
        for u in range(4):
            out[32 * u : 32 * u + NLAB] = oh
            out[32 * u + NLAB] = brow
        return out.astype(bf)

    def pack_ext(x):   # [RPC, C] fp32 -> [C, RPC] bf16
        return np.ascontiguousarray((x * SQ).T).astype(bf)

    in_maps = []
    for cix in range(NCORES):
        sl = slice(cix * RPC, (cix + 1) * RPC)
        in_maps.append({
            "ext1": pack_ext(f1[sl]),
            "ext2": pack_ext(f2[sl]),
            "extmem": extmem,
            "eqanc1": np.ascontiguousarray(eq_anchor(l1[sl], pos[sl])),
            "eqanc2": np.ascontiguousarray(eq_anchor(l2[sl], pos[sl])),
            "eqmem": eqmem,
        })
    return in_maps, pos


def _finalize(results, inputs):
    g1 = np.asarray(inputs["pseudo_logits1"], np.float64)
    g2 = np.asarray(inputs["pseudo_logits2"], np.float64)

    # device partials -> S per row, ordered [core, tile, lane]
    S = np.zeros((2, N), np.float64)
    for cix, r in enumerate(results):
        o = np.asarray(r["out"], np.float64)         # [128, 6*NT]
        for b in range(2):
            ssa = o[:, b * NT : (b + 1) * NT]
            ssd = o[:, 2 * NT + b * NT : 2 * NT + (b + 1) * NT]
            ssp = o[:, 4 * NT + b * NT : 4 * NT + (b + 1) * NT]
            st = ssa + ssd + ssp                     # [128 lanes, NT]
            # row (cix*RPC + t*128 + lane) <- st[lane, t]
            S[b, cix * RPC : (cix + 1) * RPC] = st.T.reshape(RPC)

    S = np.nan_to_num(S, nan=np.inf, posinf=np.inf, neginf=0.0)
    with np.errstate(divide="ignore", over="ignore"):
        sig = 1.0 / (S + 1.0 + EPS)
        lam = -np.log(sig + EPS)                     # per-row loss term

    m1 = ((g2 > POS_THRESH) & (g1 < g2)).astype(np.float64)
    m2 = ((g1 > POS_THRESH) & (g2 < g1)).astype(np.float64)
    loss = (lam[0] * m1).sum() / (m1.sum() + 1e-12) + \
           (lam[1] * m2).sum() / (m2.sum() + 1e-12)
    return np.float32(loss)


def _run(inputs, trace=False):
    from concourse.bass_utils import run_bass_kernel_spmd

    if "nc" not in _cache:
        _cache["nc"] = _build()
    in_maps, _pos = _host_prep(inputs)
    res = run_bass_kernel_spmd(
        _cache["nc"], in_maps, list(range(NCORES)), trace=trace
    )
    return _finalize(res.results, inputs), res


def kernel(**inputs):
    out, _ = _run(inputs)
    return out


def kernel_with_profile(**inputs):
    out, res = _run(inputs, trace=True)
    return out, res


# revision 17
# speedup vs baseline: 2.1711x; 1.2060x over previous
"""DirectionalContrastiveLoss on 8 TRN2 NeuronCores (Bass/Tile), v2.

Data-parallel over the N=16384 anchor rows (2048 rows/core); the 4000-row
memory bank is replicated (padded to 4096 columns with zero features).

Device algorithm:
- PSUM holds SC*(sim - pos + B0) per [128-row tile x 4096 mem cols],
  SC = 184.664 = 2^7/ln2, B0 = 88.03 - schraudolph shift:
  * features as fp8e4m3 DoubleRow matmuls (K=256 in one pass, both sides
    scaled by sqrt(SC/TEMP) = 42.97),
  * label mask as bf16 -SC*1000*onehot(label) x onehot(mem_label)
    matmuls on per-chunk 32-row PE tile positions (4-way concurrent),
  * per-row bias SC*(B0 - pos) via a 22nd one-hot row (x valid-col
    indicator, so pad columns stay exactly 0).
- Split exp-sum: ACT exps cols [0,ACOLS) (scale=1/SC, bias=-B0) with
  accum_out; DVE turns cols [ACOLS,4096) into int16 = clamp(psum, 0,
  32512) whose bf16 bitcast IS Schraudolph exp (no scale needed - it's
  in the matmul); DVE+GpSimd reduce-sum the bitcast tile.
- Rows with sim-pos > ~89.5 saturate to huge-finite/inf -> the host's
  -log(1/(S+1+eps)+eps) clamps them to the reference's 18.42 value
  (dead rows), exactly matching the fp32 reference semantics.
Device exports per-row partial sums [128, 6*NT]; the host computes the
-log epilogue, masks, and the final scalar (exact fp64).
"""
from contextlib import ExitStack

import numpy as np
import ml_dtypes

TEMP = 0.1
POS_THRESH = 0.7
EPS = 1e-8
N, C, M, NLAB = 16384, 256, 4000, 21
MP = 4096                  # memory columns padded
NCORES = 8
RPC = N // NCORES          # 2048 rows per core
NT = RPC // 128            # 16 n-tiles per core
CHUNK = 256                # psum region granularity (DR matmul out cols)
NCH = MP // CHUNK          # 16 chunks per n-tile

SC = 128.0 / np.log(2.0)            # 184.6635
SHIFT = 0.0579                      # schraudolph centering (in ln2 units)
B0 = (127.0 - SHIFT) * np.log(2.0)  # 87.9896  (psum bias; ACT cancels it)
SQ = float(np.sqrt(SC / TEMP))      # 42.9725  (bf16 per-side scale)
CAPV = 32512.0                      # 0x7F00 -> bf16 1.66e38 (huge finite)
J = 512                             # psum chunk width (bank)

NU = 4                     # psum units of 1024 cols; unit 3 -> DVE path
UNIT = MP // NU            # 1024
BCOLS = UNIT               # cols converted by DVE (schraudolph)

_cache = {}


def _build():
    import concourse.bacc as bacc
    import concourse.tile as tile
    from concourse import mybir

    f32 = mybir.dt.float32
    bf16 = mybir.dt.bfloat16
    f8 = mybir.dt.float8e4
    i16 = mybir.dt.int16
    Alu = mybir.AluOpType
    Act = mybir.ActivationFunctionType
    X = mybir.AxisListType.X
    DR = mybir.MatmulPerfMode.DoubleRow

    nc = bacc.Bacc(None)

    # DRAM params. ext: [C, RPC] bf16 (2 K-tiles of 128 partitions).
    ext1_d = nc.declare_dram_parameter("ext1", [C, RPC], bf16, isOutput=False)
    ext2_d = nc.declare_dram_parameter("ext2", [C, RPC], bf16, isOutput=False)
    mem_d = nc.declare_dram_parameter("extmem", [C, MP], bf16, isOutput=False)
    eqa1_d = nc.declare_dram_parameter("eqanc1", [128, RPC], bf16, isOutput=False)
    eqa2_d = nc.declare_dram_parameter("eqanc2", [128, RPC], bf16, isOutput=False)
    eqm_d = nc.declare_dram_parameter("eqmem", [128, MP], bf16, isOutput=False)
    out_d = nc.declare_dram_parameter("out", [128, 8 * NT], f32, isOutput=True)

    T0C = 4  # tiles in the startup DMA chunk

    with tile.TileContext(nc) as tc, ExitStack() as ctx:
        consts = ctx.enter_context(tc.tile_pool(name="consts", bufs=1))
        psum = ctx.enter_context(tc.tile_pool(name="psum", bufs=1, space="PSUM"))
        sb = ctx.enter_context(tc.tile_pool(name="sb", bufs=2))

        # ---- resident inputs, ordered by first use ----
        # (branch 0, tile 0 needs: ext1 t<4, mem, eqa1 t<4, eqm)
        e1_k = [
            consts.tile([128, RPC], bf16, tag=f"e1_{i}", name=f"e1_{i}")
            for i in range(2)
        ]
        e2_k = [
            consts.tile([128, RPC], bf16, tag=f"e2_{i}", name=f"e2_{i}")
            for i in range(2)
        ]
        mem_k = [
            consts.tile([128, MP], bf16, tag=f"mem_{i}", name=f"mem_{i}")
            for i in range(2)
        ]
        eqa1 = consts.tile([128, RPC], bf16, tag="eqa1", name="eqa1")
        eqa2 = consts.tile([128, RPC], bf16, tag="eqa2", name="eqa2")
        eqm = consts.tile([128, MP], bf16, tag="eqm", name="eqm")

        # unit order is [3, 0, 1, 2]; load unit-3 memory columns first so
        # tile-0's first matmuls can start ~1MB sooner.
        t0c = T0C * 128
        u3 = slice(3 * UNIT, MP)
        for i in range(2):
            nc.sync.dma_start(
                out=e1_k[i][:, 0:t0c], in_=ext1_d[128 * i : 128 * i + 128, 0:t0c]
            )
            nc.sync.dma_start(
                out=mem_k[i][:, u3], in_=mem_d[128 * i : 128 * i + 128, u3]
            )
        nc.sync.dma_start(out=eqa1[:, 0:t0c], in_=eqa1_d[:, 0:t0c])
        nc.sync.dma_start(out=eqm[:, u3], in_=eqm_d[:, u3])
        for i in range(2):
            nc.sync.dma_start(
                out=mem_k[i][:, 0 : 3 * UNIT],
                in_=mem_d[128 * i : 128 * i + 128, 0 : 3 * UNIT],
            )
        nc.sync.dma_start(out=eqm[:, 0 : 3 * UNIT], in_=eqm_d[:, 0 : 3 * UNIT])
        for i in range(2):
            nc.sync.dma_start(
                out=e1_k[i][:, t0c:], in_=ext1_d[128 * i : 128 * i + 128, t0c:]
            )
        nc.sync.dma_start(out=eqa1[:, t0c:], in_=eqa1_d[:, t0c:])
        for i in range(2):
            nc.sync.dma_start(out=e2_k[i][:], in_=ext2_d[128 * i : 128 * i + 128, :])
        nc.sync.dma_start(out=eqa2[:], in_=eqa2_d[:])

        biasA = consts.tile([128, 1], f32, tag="biasA", name="biasA")
        nc.vector.memset(biasA[:], -B0)

        # SS[u] holds per-unit row sums; host adds all 4 slots.
        SS = consts.tile([128, NU, 2, NT], f32, tag="SS", name="SS")

        UORD = [3, 0, 1, 2]  # unit 3 (DVE path) first

        for b, (ekt, eqa) in enumerate([(e1_k, eqa1), (e2_k, eqa2)]):
            for t in range(NT):
                tc0 = t * 128
                pu = {
                    u: psum.tile([128, UNIT], f32, tag=f"pu{u}", name=f"pu{u}_{b}_{t}")
                    for u in UORD
                }
                for kt in range(2):
                    lhsT = ekt[kt][:, tc0 : tc0 + 128]
                    for u in UORD:
                        for j in range(2):
                            o0 = j * J
                            nc.tensor.matmul(
                                pu[u][:, o0 : o0 + J],
                                lhsT,
                                mem_k[kt][:, u * UNIT + o0 : u * UNIT + o0 + J],
                                start=(kt == 0),
                                stop=False,
                            )
                for j in range(2):
                    for u in UORD:
                        o0 = j * J
                        nc.tensor.matmul(
                            pu[u][:, o0 : o0 + J],
                            eqa[32 * u : 32 * u + NLAB + 1, tc0 : tc0 + 128],
                            eqm[
                                32 * u : 32 * u + NLAB + 1,
                                u * UNIT + o0 : u * UNIT + o0 + J,
                            ],
                            start=False,
                            stop=True,
                            tile_position=(32 * u, 0),
                        )

                # unit 3 -> DVE: int16 = clamp(psum, 0, 32512); bf16 bitcast
                # IS schraudolph exp; reduce the bitcast tile.
                ex = sb.tile([128, BCOLS], i16, tag="ex", name=f"ex{b}_{t}")
                nc.vector.tensor_scalar(
                    out=ex[:],
                    in0=pu[3][:],
                    scalar1=0.0,
                    scalar2=CAPV,
                    op0=Alu.max,
                    op1=Alu.min,
                )
                exb = ex[:].bitcast(bf16)
                nc.vector.reduce_sum(
                    out=SS[:, 3, b, t : t + 1], in_=exb[:], axis=X
                )

                # units 0-2 -> ACT: exp((psum/SC) - B0), accum row sum
                for u in (0, 1, 2):
                    nc.scalar.activation(
                        out=pu[u][:],
                        in_=pu[u][:],
                        func=Act.Exp,
                        bias=biasA[:],
                        scale=float(1.0 / SC),
                        accum_out=SS[:, u, b, t : t + 1],
                    )

        nc.sync.dma_start(
            out=out_d[:], in_=SS[:].rearrange("p u b t -> p (u b t)")
        )

    nc.finalize()
    return nc


def _host_prep(inputs):
    bf = ml_dtypes.bfloat16
    f8 = ml_dtypes.float8_e4m3
    f1 = np.ascontiguousarray(np.asarray(inputs["output_feat1"], np.float32))
    f2 = np.ascontiguousarray(np.asarray(inputs["output_feat2"], np.float32))
    l1 = np.asarray(inputs["pseudo_label1"], np.int32)
    l2 = np.asarray(inputs["pseudo_label2"], np.int32)
    ul1 = np.asarray(inputs["output_ul1"], np.float32)
    ul2 = np.asarray(inputs["output_ul2"], np.float32)
    i1 = np.asarray(inputs["selected_idx1"], np.int64)
    i2 = np.asarray(inputs["selected_idx2"], np.int64)

    b, c, h, w = ul1.shape
    u1 = ul1.transpose(0, 2, 3, 1).reshape(b * h * w, c)
    u2 = ul2.transpose(0, 2, 3, 1).reshape(b * h * w, c)
    mem = np.concatenate([u1[i1], u2[i2]], axis=0)               # [M, C]
    memlab = np.concatenate([l1[i1], l2[i2]], axis=0)            # [M]

    pos = (f1 * f2).sum(axis=1, dtype=np.float64) / TEMP         # [N] exact

    extmem = np.zeros((C, MP), np.float32)
    extmem[:, :M] = mem.T * SQ
    extmem = extmem.astype(bf)                                   # [C, MP]

    # mask memory side: rows 32u+i = onehot(memlab==i); row 32u+21 = valid
    lab_eye = np.arange(NLAB, dtype=np.int32)
    eqmem = np.zeros((128, MP), np.float32)
    oh_mem = (memlab[None, :] == lab_eye[:, None]).astype(np.float32)
    for u in range(4):
        eqmem[32 * u : 32 * u + NLAB, :M] = oh_mem
        eqmem[32 * u + NLAB, :M] = 1.0
    eqmem = eqmem.astype(bf)

    def eq_anchor(lab, pos_sl):
        out = np.zeros((128, lab.shape[0]), np.float32)
        oh = (-SC * 1000.0) * (lab[None, :] == lab_eye[:, None])
        brow = SC * (B0 - pos_sl)
        for u in range(4):
            out[32 * u : 32 * u + NLAB] = oh
            out[32 * u + NLAB] = brow
        return out.astype(bf)

    def pack_ext(x):   # [RPC, C] fp32 -> [C, RPC] bf16
        return np.ascontiguousarray((x * SQ).T).astype(bf)

    in_maps = []
    for cix in range(NCORES):
        sl = slice(cix * RPC, (cix + 1) * RPC)
        in_maps.append({
            "ext1": pack_ext(f1[sl]),
            "ext2": pack_ext(f2[sl]),
            "extmem": extmem,
            "eqanc1": np.ascontiguousarray(eq_anchor(l1[sl], pos[sl])),
            "eqanc2": np.ascontiguousarray(eq_anchor(l2[sl], pos[sl])),
            "eqmem": eqmem,
        })
    return in_maps, pos


def _finalize(results, inputs):
    g1 = np.asarray(inputs["pseudo_logits1"], np.float64)
    g2 = np.asarray(inputs["pseudo_logits2"], np.float64)

    # device partials -> S per row, ordered [core, tile, lane]
    S = np.zeros((2, N), np.float64)
    for cix, r in enumerate(results):
        o = np.asarray(r["out"], np.float64).reshape(128, NU, 2, NT)
        st = o.sum(axis=1)                           # [128 lanes, 2, NT]
        for b in range(2):
            # row (cix*RPC + t*128 + lane) <- st[lane, b, t]
            S[b, cix * RPC : (cix + 1) * RPC] = st[:, b].T.reshape(RPC)

    S = np.nan_to_num(S, nan=np.inf, posinf=np.inf, neginf=0.0)
    with np.errstate(divide="ignore", over="ignore"):
        sig = 1.0 / (S + 1.0 + EPS)
        lam = -np.log(sig + EPS)                     # per-row loss term

    m1 = ((g2 > POS_THRESH) & (g1 < g2)).astype(np.float64)
    m2 = ((g1 > POS_THRESH) & (g2 < g1)).astype(np.float64)
    loss = (lam[0] * m1).sum() / (m1.sum() + 1e-12) + \
           (lam[1] * m2).sum() / (m2.sum() + 1e-12)
    return np.float32(loss)


def _run(inputs, trace=False):
    from concourse.bass_utils import run_bass_kernel_spmd

    if "nc" not in _cache:
        _cache["nc"] = _build()
    in_maps, _pos = _host_prep(inputs)
    res = run_bass_kernel_spmd(
        _cache["nc"], in_maps, list(range(NCORES)), trace=trace
    )
    return _finalize(res.results, inputs), res


def kernel(**inputs):
    out, _ = _run(inputs)
    return out


def kernel_with_profile(**inputs):
    out, res = _run(inputs, trace=True)
    return out, res
